# revision 52
# baseline (speedup 1.0000x reference)
"""Trainium2 Bass kernel for CausalSelfAttention with external-memory prefix.

Problem shapes (hardcoded): B=2, T=2048, C=1024, H=16, HD=64, MEM=256.
Sharding: 8 cores = 2 (batch) x 4 (head groups of 4 heads).
Each core computes, for its batch b and heads [4g, 4g+4):
  qkv slice -> flash attention (mem prefix + causal) -> partial y @ W_proj rows.
Host unshards by summing the 4 head-group partials per batch (the g==0 core
adds b_proj via its bias input; other cores get zeros).

Device algorithm per core:
  - x^T built via PE transposes (contraction over C needs C on partitions).
  - qT/kT = W^T x^T (channels on partitions); v = x W_v (t on partitions),
    written strided into a [V | 1] buffer so PV fuses the softmax rowsum.
  - scores computed transposed: S^T[s, t] = k^T_slice^T @ q^T  (K=64).
  - P^T = exp(0.125 * S^T) on ScalarE (scores bounded ~|5.3| after scaling,
    so no max-subtraction is needed; validated vs reference); causal masking
    is a multiplicative 0/1 mask on P^T (DVE) for diagonal blocks.
  - y^T/denominator: psum[65, 512] += [V|1]^T @ P^T accumulated over s-blocks.
  - denominators: per (head, t-block): row -> DRAM -> [128, 4] gather ->
    reciprocal (DVE) -> DRAM scatter -> row fetch -> gpsimd
    partition-broadcast -> multiply into y^T.
  - out = y^T^T @ W_proj_rows + b_proj (bias seeded with a K=1 matmul).

Matmul operands use dt.float32r (single-pass PE mode, 4x faster than fp32,
measured ~1.6e-4 matmul rel err). fp32r inputs must be produced by a compute
engine ("rounded"), so DMA'd weights take a DVE pass-through copy.
"""

import numpy as np
from contextlib import ExitStack

import concourse.bass as bass
import concourse.tile as tile
from concourse import mybir
from concourse import bacc
from concourse import bass_utils

FP32 = mybir.dt.float32
R32 = mybir.dt.float32r
AF = mybir.ActivationFunctionType
ALU = mybir.AluOpType

P = 128
T = 2048
C = 1024
HPC = 4        # heads per core
HD = 64
MEM = 256
S = MEM + T    # 2304
NST = S // P   # 18 s-tiles (0-1 mem, 2-17 causal)
NEG = -1.0e9


def build_nc() -> bass.Bass:
    nc = bacc.Bacc(
        "TRN2", target_bir_lowering=False, debug=False, num_devices=8
    )
    x_d = nc.dram_tensor("x", (T, C), FP32, kind="ExternalInput").ap()
    # host-built constants: [masks(4x512) | identity(128) | ones(260)]
    cst_d = nc.dram_tensor("cst", (P, 2436), FP32, kind="ExternalInput").ap()
    wqk_d = nc.dram_tensor("wqk", (C, 512), FP32, kind="ExternalInput").ap()
    wv_d = nc.dram_tensor("wv", (C, 256), FP32, kind="ExternalInput").ap()
    bqk_d = nc.dram_tensor("bqk", (P, 4), FP32, kind="ExternalInput").ap()
    bv_d = nc.dram_tensor("bv", (1, 256), FP32, kind="ExternalInput").ap()
    mem_d = nc.dram_tensor("mem", (MEM, 256), FP32, kind="ExternalInput").ap()
    wp_d = nc.dram_tensor("wp", (256, C), FP32, kind="ExternalInput").ap()
    bp_d = nc.dram_tensor("bp", (1, C), FP32, kind="ExternalInput").ap()
    out_d = nc.dram_tensor("out", (T, C), FP32, kind="ExternalOutput").ap()
    # DRAM scratch for the softmax-denominator partition shuffle.
    dscr = nc.dram_tensor("dscr", (16, 512), FP32, kind="Internal").ap()
    rscr = nc.dram_tensor("rscr", (1, 16 * 512), FP32, kind="Internal").ap()

    with tile.TileContext(nc) as tc, ExitStack() as ctx:
        const = ctx.enter_context(tc.tile_pool(name="const", bufs=1))
        big = ctx.enter_context(tc.tile_pool(name="big", bufs=1))
        # single recycled staging slot for fp32 -> fp32r conversions
        stage = ctx.enter_context(tc.tile_pool(name="stage", bufs=1))

        # ---- constants (all host-built, DMA'd, converted on DVE) ----
        cst_sb = const.tile([P, 2436], FP32)
        # identity+ones first (tiny; first transposes depend on it)
        nc.sync.dma_start(cst_sb[:, 2048:2436], cst_d[:, 2048:2436])
        ident = cst_sb[:, 2048:2176]
        ones_f = cst_sb[:, 2176:2436]
        ones1 = const.tile([1, P], R32)
        nc.vector.tensor_copy(out=ones1, in_=ones_f[:1, :P])
        bqk_sb = const.tile([P, 4], FP32)
        bv_sb = const.tile([1, 256], R32)
        bp_sb = const.tile([1, C], R32)
        masks = const.tile([P, 4, 512], R32)

        def emit_small_consts():
            # masks[k, j, tf] = 0 if tf >= k + 128*j else -1e9 (S^T layout)
            nc.sync.dma_start(cst_sb[:, :2048], cst_d[:, :2048])
            nc.vector.tensor_copy(
                out=masks,
                in_=cst_sb[:, :2048].rearrange("p (j n) -> p j n", n=512),
            )
            bvp_f = stage.tile([P, 8, 512], FP32, tag="stage", name="bvp_f")
            nc.sync.dma_start(bvp_f[:1, 1, :512], bp_d[:, :512])
            nc.sync.dma_start(bvp_f[:1, 2, :512], bp_d[:, 512:])
            nc.vector.tensor_copy(
                out=bp_sb.rearrange("o (a n) -> o a n", a=2),
                in_=bvp_f[:1, 1:3, :512],
            )

        # ---- persistent buffers ----
        # qkT rows per 128-tile: [q h0h1, q h2h3, k h0h1, k h2h3]
        qkT = big.tile([P, 4, T], R32)
        # [V | 1] per head, 65 columns each, s on partitions (18 s-tiles)
        vones = big.tile([P, NST, 65 * HPC], R32)
        for st in range(NST):
            nc.vector.tensor_copy(out=vones[:, st, :], in_=ones_f)
        # mem keys transposed: channels (4 heads x 64) on partitions
        kTm = big.tile([P, 2, MEM], R32)

        # =========== phase A: x^T, v, mem, qkT ===========
        with (
            tc.tile_pool(name="pA", bufs=1) as pA,
            tc.tile_pool(name="pAt", bufs=3) as pAt,
            tc.tile_pool(name="pAp", bufs=4, space="PSUM") as pAp,
        ):
            xT = pA.tile([P, 8, T], R32)
            wqk_sb = pA.tile([P, 8, 512], R32)
            wv_sb = pA.tile([P, 8, 256], R32)
            memsb = pAt.tile([P, 2, 256], FP32, tag="memsb", bufs=1)

            wv_f = stage.tile([P, 8, 512], FP32, tag="stage", name="wv_f")
            bvp_f2 = pAt.tile([1, 256], FP32, tag="bvp2", bufs=1, name="bvp_f2")

            def emit_v_weights():
                # small bias DMAs + wv (needed by wave-0 v matmuls, emitted
                # after the first x wave so the first transposes aren't
                # starved)
                nc.sync.dma_start(bqk_sb, bqk_d)
                nc.sync.dma_start(bvp_f2, bv_d)
                nc.vector.tensor_copy(out=bv_sb, in_=bvp_f2)
                nc.sync.dma_start(
                    wv_f[:, :, :256], wv_d.rearrange("(ko p) n -> p ko n", p=P)
                )
                nc.vector.tensor_copy(out=wv_sb, in_=wv_f[:, :, :256])

            # x^T via PE transposes (4 per psum bank), v interleaved per wave
            for tq in range(4):
                xw = []
                for half in range(2):
                    for i in range(4):
                        xr = pAt.tile([P, 512], FP32, tag="xr", bufs=8, name="xr")
                        r0 = (tq * 4 + i) * P
                        nc.sync.dma_start(
                            xr, x_d[r0 : r0 + P, half * 512 : (half + 1) * 512]
                        )
                        xw.append(xr)
                if tq == 0:
                    emit_v_weights()
                for ct in range(8):
                    ps = pAp.tile([P, 512], FP32, tag="ps")
                    for i in range(4):
                        nc.tensor.transpose(
                            ps[:, i * P : (i + 1) * P],
                            xw[(ct // 4) * 4 + i][:, (ct % 4) * P : (ct % 4 + 1) * P],
                            ident,
                        )
                    if ct % 2 == 0:
                        nc.scalar.copy(
                            out=xT[:, ct, tq * 512 : (tq + 1) * 512], in_=ps
                        )
                    else:
                        nc.vector.tensor_copy(
                            out=xT[:, ct, tq * 512 : (tq + 1) * 512], in_=ps
                        )
                # v = x @ W_v + b_v for this wave's t-tiles, strided into vones
                for tt in range(4 * tq, 4 * tq + 4):
                    psv = pAp.tile([P, 256], FP32, tag="ps", name="psv")
                    nc.tensor.matmul(
                        psv, lhsT=ones1, rhs=bv_sb, start=True, stop=False
                    )
                    for ct in range(8):
                        nc.tensor.matmul(
                            psv,
                            lhsT=xT[:, ct, tt * P : (tt + 1) * P],
                            rhs=wv_sb[:, ct, :],
                            start=False,
                            stop=(ct == 7),
                        )
                    nc.vector.tensor_copy(
                        out=vones[:, 2 + tt, :].rearrange("p (h e) -> p h e", e=65)[
                            :, :, :HD
                        ],
                        in_=psv.rearrange("p (h e) -> p h e", e=HD),
                    )

            # mem prefix: V rows into vones, keys transposed into kTm
            nc.sync.dma_start(memsb, mem_d.rearrange("(o p) n -> p o n", p=P))
            for o in range(2):
                nc.vector.tensor_copy(
                    out=vones[:, o, :].rearrange("p (h e) -> p h e", e=65)[
                        :, :, :HD
                    ],
                    in_=memsb[:, o, :].rearrange("p (h e) -> p h e", e=HD),
                )
                for j in range(2):
                    pst = pAp.tile([P, P], FP32, tag="ps")
                    nc.tensor.transpose(
                        pst, memsb[:, o, j * P : (j + 1) * P], ident
                    )
                    nc.vector.tensor_copy(
                        out=kTm[:, j, o * P : (o + 1) * P], in_=pst
                    )

            # qkT = W_qk^T @ x^T + b (channels on partitions); weight DMA
            # split per column-tile so mt0/mt2 (heads 0,1) arrive first
            wqk_f = stage.tile([P, 8, 512], FP32, tag="stage", name="wqk_f")

            def emit_qkT(mt):
                nc.sync.dma_start(
                    wqk_f[:, :, mt * P : (mt + 1) * P],
                    wqk_d[:, mt * P : (mt + 1) * P].rearrange(
                        "(ko p) n -> p ko n", p=P
                    ),
                )
                nc.vector.tensor_copy(
                    out=wqk_sb[:, :, mt * P : (mt + 1) * P],
                    in_=wqk_f[:, :, mt * P : (mt + 1) * P],
                )
                for tb in range(4):
                    psq = pAp.tile([P, 512], FP32, tag="ps", name="psq")
                    for ct in range(8):
                        nc.tensor.matmul(
                            psq,
                            lhsT=wqk_sb[:, ct, mt * P : (mt + 1) * P],
                            rhs=xT[:, ct, tb * 512 : (tb + 1) * 512],
                            start=(ct == 0),
                            stop=(ct == 7),
                        )
                    nc.vector.tensor_scalar_add(
                        qkT[:, mt, tb * 512 : (tb + 1) * 512],
                        psq,
                        bqk_sb[:, mt : mt + 1],
                    )

            emit_qkT(0)
            emit_qkT(2)
            emit_qkT(1)
            emit_qkT(3)
            emit_small_consts()

        # =========== phase B: attention + proj ===========
        with (
            tc.tile_pool(name="pB", bufs=1) as pB,
            tc.tile_pool(name="pBt", bufs=4) as pBt,
            tc.tile_pool(name="yscp", bufs=3) as yscp,
            tc.tile_pool(name="pBp", bufs=3, space="PSUM") as pBp,
            tc.tile_pool(name="pxp", bufs=1, space="PSUM") as pxp,
            tc.tile_pool(name="pyp", bufs=2, space="PSUM") as pyp,
        ):
            yTt = pB.tile([P, 2, T], FP32)
            yTs = pB.tile([P, 2, T], R32)
            wp_f = stage.tile([P, 8, 512], FP32, tag="stage", name="wp_f")
            for ko in range(2):
                nc.sync.dma_start(
                    wp_f[:, ko * 2 : (ko + 1) * 2, :],
                    wp_d[ko * P : (ko + 1) * P, :].rearrange(
                        "p (nb n) -> p nb n", n=512
                    ),
                )
            wp_sb = pB.tile([P, 2, C], R32)
            nc.vector.tensor_copy(
                out=wp_sb.rearrange("p ko (nb n) -> p ko nb n", n=512),
                in_=wp_f[:, :4, :].rearrange("p (ko nb) n -> p ko nb n", nb=2),
            )
            dcol = pB.tile([P, 64], FP32)
            rcol = pB.tile([P, 64], FP32)

            # out = yT^T @ W_proj + b_proj for one tb's four t-tiles;
            # interleaved into head 3's loop so the proj hides the last
            # denominator chains
            def emit_proj_tb(tbp):
                for tt in range(4 * tbp, 4 * tbp + 4):
                    for nb in range(2):
                        psp = pyp.tile([P, 512], FP32, tag="psy", name="psp")
                        nc.tensor.matmul(
                            psp,
                            lhsT=ones1,
                            rhs=bp_sb[:, nb * 512 : (nb + 1) * 512],
                            start=True,
                            stop=False,
                        )
                        for kt in range(2):
                            nc.tensor.matmul(
                                psp,
                                lhsT=yTs[:, kt, tt * P : (tt + 1) * P],
                                rhs=wp_sb[:, kt, nb * 512 : (nb + 1) * 512],
                                start=False,
                                stop=(kt == 1),
                            )
                        osb = pBt.tile([P, 512], FP32, tag="osb")
                        if (tt + nb) % 2 == 0:
                            nc.scalar.copy(out=osb, in_=psp)
                        else:
                            nc.vector.tensor_copy(out=osb, in_=psp)
                        nc.sync.dma_start(
                            out_d[tt * P : (tt + 1) * P, nb * 512 : (nb + 1) * 512],
                            osb,
                        )

            for h in range(HPC):
                base = HD * (h % 2)
                qt_idx = h // 2
                kt_idx = 2 + h // 2
                for tb in range(4):
                    n_st = 6 + 4 * tb
                    n_pair = n_st // 2
                    psy = pyp.tile([65, 512], FP32, tag="psy", name="psy")
                    qT_slice = qkT[base : base + HD, qt_idx, tb * 512 : (tb + 1) * 512]
                    for pr in range(n_pair):
                        pss = pBp.tile([P, 1024], FP32, tag="ps", name="pss")
                        for half in range(2):
                            st = 2 * pr + half
                            half_sl = pss[:, half * 512 : (half + 1) * 512]
                            if st < 2:
                                kT_slice = kTm[
                                    base : base + HD, qt_idx, st * P : (st + 1) * P
                                ]
                            else:
                                kT_slice = qkT[
                                    base : base + HD,
                                    kt_idx,
                                    (st - 2) * P : (st - 1) * P,
                                ]
                            nc.tensor.matmul(
                                half_sl,
                                lhsT=kT_slice,
                                rhs=qT_slice,
                                start=True,
                                stop=True,
                            )
                        pt = pBt.tile([P, 1024], R32, tag="pt", bufs=4, name="pt")
                        nc.scalar.activation(pt, pss, AF.Exp, scale=0.125)
                        for half in range(2):
                            # diagonal blocks: multiplicative 0/1 mask on the
                            # exp'd probabilities (DVE) instead of a -1e9
                            # matmul accumulation (PE is the bottleneck)
                            st = 2 * pr + half
                            diag_j = st - 2 - 4 * tb
                            if diag_j >= 0:
                                nc.vector.tensor_mul(
                                    out=pt[:, half * 512 : (half + 1) * 512],
                                    in0=pt[:, half * 512 : (half + 1) * 512],
                                    in1=masks[:, diag_j, :],
                                )
                            nc.tensor.matmul(
                                psy,
                                lhsT=vones[:, st, h * 65 : (h + 1) * 65],
                                rhs=pt[:, half * 512 : (half + 1) * 512],
                                start=(st == 0),
                                stop=(st == n_st - 1),
                            )
                    ysc = yscp.tile([65, 512], FP32, tag="ysc", name="ysc")
                    nc.vector.tensor_copy(out=ysc, in_=psy)
                    c_idx = h * 4 + tb
                    nc.sync.dma_start(
                        yTt[base : base + HD, h // 2, tb * 512 : (tb + 1) * 512],
                        ysc[:HD, :],
                    )
                    nc.sync.dma_start(dscr[c_idx : c_idx + 1, :], ysc[HD:65, :])

                    # per-(h,tb) denominator chain (overlaps next tb's
                    # attention): gather -> reciprocal -> scatter -> row
                    # fetch -> gpsimd partition-broadcast -> scale
                    cs = 4 * c_idx
                    nc.sync.dma_start(
                        dcol[:, cs : cs + 4],
                        dscr[c_idx, :].rearrange("(p j) -> p j", p=P),
                    )
                    nc.vector.reciprocal(
                        rcol[:, cs : cs + 4], dcol[:, cs : cs + 4]
                    )
                    nc.sync.dma_start(
                        rscr[:, c_idx * 512 : (c_idx + 1) * 512].rearrange(
                            "o (p j) -> (o p) j", p=P
                        ),
                        rcol[:, cs : cs + 4],
                    )
                    rr_f = pBt.tile([1, 512], FP32, tag="rrf", bufs=3, name="rr_f")
                    nc.sync.dma_start(
                        rr_f, rscr[:, c_idx * 512 : (c_idx + 1) * 512]
                    )
                    bt = pBt.tile([P, 512], FP32, tag="bt", bufs=3, name="bt")
                    nc.gpsimd.partition_broadcast(bt, rr_f)
                    nc.vector.tensor_mul(
                        out=yTs[base : base + HD, h // 2, tb * 512 : (tb + 1) * 512],
                        in0=yTt[base : base + HD, h // 2, tb * 512 : (tb + 1) * 512],
                        in1=bt[base : base + HD, :],
                    )
            for tbp in range(4):
                emit_proj_tb(tbp)

    nc.compile()
    return nc


def _build_cst() -> np.ndarray:
    kk = np.arange(P, dtype=np.int64)[:, None]
    tf = np.arange(512, dtype=np.int64)[None, :]
    masks = np.concatenate(
        [
            np.where(tf >= kk + 128 * j, 1.0, 0.0).astype(np.float32)
            for j in range(4)
        ],
        axis=1,
    )  # [128, 2048], multiplicative post-exp masks
    ident = np.eye(P, dtype=np.float32)
    ones = np.ones((P, 260), np.float32)
    return np.concatenate([masks, ident, ones], axis=1)


_CST = _build_cst()


def shard_inputs(inputs: dict) -> list:
    x = np.asarray(inputs["x"], dtype=np.float32)
    em = np.asarray(inputs["ext_mem"], dtype=np.float32)
    wa = np.asarray(inputs["W_attn"], dtype=np.float32)
    ba = np.asarray(inputs["b_attn"], dtype=np.float32)
    wp = np.asarray(inputs["W_proj"], dtype=np.float32)
    bp = np.asarray(inputs["b_proj"], dtype=np.float32)

    in_maps = []
    for c in range(8):
        b, g = c // 4, c % 4
        lo = g * 256
        wqk = np.concatenate(
            [wa[:, lo : lo + 256], wa[:, 1024 + lo : 1024 + lo + 256]], axis=1
        )
        bqk = np.concatenate(
            [ba[lo : lo + 256], ba[1024 + lo : 1024 + lo + 256]]
        ).reshape(4, P).T
        in_maps.append(
            {
                "x": np.ascontiguousarray(x[b]),
                "cst": _CST,
                "wqk": np.ascontiguousarray(wqk),
                "wv": np.ascontiguousarray(wa[:, 2048 + lo : 2048 + lo + 256]),
                "bqk": np.ascontiguousarray(bqk),
                "bv": np.ascontiguousarray(ba[2048 + lo : 2048 + lo + 256][None]),
                "mem": np.ascontiguousarray(em[b][:, lo : lo + 256]),
                "wp": np.ascontiguousarray(wp[lo : lo + 256, :]),
                "bp": np.ascontiguousarray(
                    bp[None] if g == 0 else np.zeros((1, C), np.float32)
                ),
            }
        )
    return in_maps


_CACHE: dict = {}


def run_sharded(inputs: dict, trace: bool = False):
    """Returns (full_output [2, T, C], exec_time_ns or None)."""
    nc = _CACHE.get("nc")
    if nc is None:
        nc = build_nc()
        _CACHE["nc"] = nc
    in_maps = shard_inputs(inputs)
    res = bass_utils.run_bass_kernel_spmd(
        nc, in_maps, core_ids=list(range(8)), trace=trace
    )
    parts = [res.results[c]["out"] for c in range(8)]
    full = np.stack(
        [
            parts[0] + parts[1] + parts[2] + parts[3],
            parts[4] + parts[5] + parts[6] + parts[7],
        ]
    ).astype(np.float32)
    return full, res.exec_time_ns


def kernel(**inputs) -> np.ndarray:
    out, _ = run_sharded(inputs, trace=False)
    return out
